# revision 15
# baseline (speedup 1.0000x reference)
"""Multi-head self-attention (RoPE + bias + pad-mask + head-mean metric) on 8 TRN2 cores.

Sharding: (batch, query-half) -> one core each: core c handles batch c//2,
queries [c%2*512, (c%2+1)*512). No collectives; host does pure slicing/concat.

Device layout (per core):
  - projections computed transposed: qT/kT [j, s] via lhsT=W^T-tile, rhs=x^T
  - rotary via "swap projection": second matmul chain against sign-permuted
    weight columns, then 2 muls (cos/sin tables) + add on DVE
  - scores computed transposed [k, q] (c=64 matmuls), softmax without
    max-subtraction (scores are bounded), pad mask + attn bias applied
    multiplicatively: p = exp(scores) * eb where eb = exp(bias) (0 if padded),
    streamed as bf16 via DMA-transpose
  - denominators via ones-column augmented V (row 64 of the AV psum), divided
    in via reciprocal + DRAM-round-trip partition broadcast
  - o_proj directly from the transposed attention output (lhsT), metric
    (head-mean of k_rot) via a constant averaging matrix on the PE
"""

import os
import numpy as np
import ml_dtypes

B, S, D, H, HD = 4, 1024, 1024, 16, 64
SQ = S // 2          # queries per core
NCORES = 8
NT = D // 128        # contraction tiles
BF = ml_dtypes.bfloat16

_prog_cache = {}
_last_results = None  # BassKernelResults of the most recent run (for test.py)


def _build_program():
    import concourse.bass as bass
    import concourse.tile as tile
    from concourse import bacc, mybir

    f32 = mybir.dt.float32
    bf16 = mybir.dt.bfloat16
    AF = mybir.ActivationFunctionType

    nc = bacc.Bacc("TRN2", target_bir_lowering=False, debug=False)

    def din(name, shape, dt=bf16):
        return nc.dram_tensor(name, shape, dt, kind="ExternalInput").ap()

    xq_h = din("xq", [D, SQ])
    wq_h = din("wq", [D, D])
    wk_h = din("wk", [D, D])
    wv_h = din("wv", [D, D]); wo_h = din("wo", [D, D])
    cq_h = din("cq", [128, SQ], f32); sq_h = din("sq", [128, SQ], f32)
    ck_h = din("ck", [128, SQ], f32); sk_h = din("sk", [128, SQ], f32)
    eb_h = din("eb", [H, S, SQ])
    mpat_h = din("mpat", [128, HD])

    outp_h = nc.dram_tensor("outp", [SQ, D], f32, kind="ExternalOutput").ap()
    metT_h = nc.dram_tensor("metT", [HD, S], f32, kind="ExternalOutput").ap()

    dscr = nc.dram_tensor("dscr", [H, SQ], f32).ap()  # rden bounce buffer
    cin_k = nc.dram_tensor("cin_k", [D, SQ], bf16).ap()
    cout_k = nc.dram_tensor("cout_k", [2 * D, SQ], bf16).ap()
    cin_v = nc.dram_tensor("cin_v", [SQ, H * (HD + 1)], bf16).ap()
    cout_v = nc.dram_tensor("cout_v", [S, H * (HD + 1)], bf16).ap()

    with tile.TileContext(nc) as tc:
        with tc.tile_pool(name="keep", bufs=1) as keep:
            # persistent across phases
            wo = [keep.tile([128, D], bf16, tag=f"wo{i}", name=f"wo{i}") for i in range(NT)]
            qrot = [keep.tile([128, SQ], bf16, tag=f"qr{i}", name=f"qr{i}") for i in range(NT)]
            krot = [keep.tile([128, S], bf16, tag=f"kr{i}", name=f"kr{i}") for i in range(NT)]
            vaug = [keep.tile([128, H * (HD + 1)], bf16, tag=f"va{i}", name=f"va{i}") for i in range(NT)]
            outT = [keep.tile([128, SQ], bf16, tag=f"ot{i}", name=f"ot{i}") for i in range(NT)]
            mpat = keep.tile([128, HD], bf16, tag="mpat")
            nc.sync.dma_start(out=mpat, in_=mpat_h)
            for i in range(NT):
                nc.sync.dma_start(out=wo[i], in_=wo_h[i * 128:(i + 1) * 128, :])

            # ---------------- phase 1: projections + rotary ----------------
            with tc.tile_pool(name="ph1", bufs=1) as ph1, \
                 tc.tile_pool(name="t1", bufs=4) as t1p, \
                 tc.tile_pool(name="ppq", bufs=2, space="PSUM") as ppq, \
                 tc.tile_pool(name="ppk", bufs=2, space="PSUM") as ppk, \
                 tc.tile_pool(name="ppv", bufs=1, space="PSUM") as ppv:
                xq = [ph1.tile([128, SQ], bf16, tag=f"xq{i}", name=f"xq{i}") for i in range(NT)]
                krot_h = [ph1.tile([128, SQ], bf16, tag=f"krh{i}", name=f"krh{i}") for i in range(NT)]
                vau_h = [ph1.tile([128, H * (HD + 1)], bf16, tag=f"vah{i}", name=f"vah{i}") for i in range(4)]
                wq = [ph1.tile([128, D], bf16, tag=f"wq{i}", name=f"wq{i}") for i in range(NT)]
                wk = [ph1.tile([128, D], bf16, tag=f"wk{i}", name=f"wk{i}") for i in range(NT)]
                wv = [ph1.tile([128, D], bf16, tag=f"wv{i}", name=f"wv{i}") for i in range(NT)]
                for i in range(NT):
                    sl = slice(i * 128, (i + 1) * 128)
                    nc.sync.dma_start(out=xq[i], in_=xq_h[sl, :])
                    nc.sync.dma_start(out=wq[i], in_=wq_h[sl, :])
                    nc.scalar.dma_start(out=wk[i], in_=wk_h[sl, :])
                    nc.gpsimd.dma_start(out=wv[i], in_=wv_h[sl, :])
                import concourse.bass as _bass
                cq = ph1.tile([128, SQ], f32, tag="cq")
                sq = ph1.tile([128, SQ], f32, tag="sq")
                ck = ph1.tile([128, SQ], f32, tag="ck")
                sk = ph1.tile([128, SQ], f32, tag="sk")
                nc.sync.dma_start(out=cq, in_=cq_h)
                nc.sync.dma_start(out=sq, in_=sq_h)
                nc.sync.dma_start(out=ck, in_=ck_h)
                nc.sync.dma_start(out=sk, in_=sk_h)

                def v_tile(lst):
                    ss = slice(lst * 128, (lst + 1) * 128)
                    ones_ap = _bass.AP(
                        tensor=vau_h[lst].tensor,
                        offset=vau_h[lst].offset + HD,
                        ap=[vau_h[lst].ap[0], [HD + 1, H], [1, 1]],
                    )
                    nc.vector.memset(ones_ap, 1.0)
                    psv = ppv.tile([128, 2, 512], f32, tag="psv", name=f"psv{lst}")
                    for dt in range(NT):
                        for jh in range(2):
                            nc.tensor.matmul(psv[:, jh, :], lhsT=xq[dt][:, ss],
                                             rhs=wv[dt][:, jh * 512:(jh + 1) * 512],
                                             start=(dt == 0), stop=(dt == NT - 1))
                    dest = _bass.AP(
                        tensor=vau_h[lst].tensor,
                        offset=vau_h[lst].offset,
                        ap=[vau_h[lst].ap[0], [HD + 1, H], [1, HD]],
                    )
                    nc.scalar.copy(out=dest, in_=psv.rearrange("p c b -> p (c b)").rearrange("p (a b) -> p a b", a=H))

                def swap_dma(dst, src, width):
                    # dst[p] = src[p+-32 within each 64 block], via 4 shifted DMAs
                    for i, (a, bb) in enumerate(((0, 32), (32, 0), (64, 96), (96, 64))):
                        eng = nc.scalar if i % 2 == 0 else nc.gpsimd
                        eng.dma_start(out=dst[a:a + 32, :width],
                                      in_=src[bb:bb + 32, :width])

                for jt in range(NT):
                    js = slice(jt * 128, (jt + 1) * 128)
                    # q projection (m=j-tile, n=q), rotary via evac + swap-DMA
                    psq = ppq.tile([128, SQ], f32, tag="psq")
                    for dt in range(NT):
                        nc.tensor.matmul(psq, lhsT=wq[dt][:, js], rhs=xq[dt],
                                         start=(dt == 0), stop=(dt == NT - 1))
                    qsb = t1p.tile([128, S], f32, tag="tmp")
                    qsw = t1p.tile([128, S], f32, tag="tmp")
                    nc.scalar.copy(out=qsb[:, :SQ], in_=psq)
                    swap_dma(qsw, qsb, SQ)
                    t1 = t1p.tile([128, S], f32, tag="tmp")
                    t2 = t1p.tile([128, S], f32, tag="tmp")
                    nc.vector.tensor_mul(out=t1[:, :SQ], in0=qsb[:, :SQ], in1=cq)
                    nc.vector.tensor_mul(out=t2[:, :SQ], in0=qsw[:, :SQ], in1=sq)
                    nc.vector.tensor_add(out=qrot[jt], in0=t1[:, :SQ], in1=t2[:, :SQ])

                    # k projection for own s-half only
                    psk = ppk.tile([128, SQ], f32, tag="psk")
                    for dt in range(NT):
                        nc.tensor.matmul(psk, lhsT=wk[dt][:, js], rhs=xq[dt],
                                         start=(dt == 0), stop=(dt == NT - 1))
                    ksb = t1p.tile([128, S], f32, tag="tmp")
                    ksw = t1p.tile([128, S], f32, tag="tmp")
                    nc.scalar.copy(out=ksb[:, :SQ], in_=psk)
                    swap_dma(ksw, ksb, SQ)
                    t3 = t1p.tile([128, S], f32, tag="tmp")
                    t4 = t1p.tile([128, S], f32, tag="tmp")
                    nc.vector.tensor_mul(out=t3[:, :SQ], in0=ksb[:, :SQ], in1=ck)
                    nc.vector.tensor_mul(out=t4[:, :SQ], in0=ksw[:, :SQ], in1=sk)
                    nc.vector.tensor_add(out=krot_h[jt], in0=t3[:, :SQ], in1=t4[:, :SQ])
                    nc.scalar.dma_start(out=cin_k[jt * 128:(jt + 1) * 128, :],
                                        in_=krot_h[jt])
                    if jt < 4:
                        v_tile(jt)
                        nc.sync.dma_start(
                            out=cin_v[jt * 128:(jt + 1) * 128, :], in_=vau_h[jt])

                nc.gpsimd.collective_compute(
                    "AllGather", mybir.AluOpType.bypass,
                    replica_groups=[[0, 1], [2, 3], [4, 5], [6, 7]],
                    ins=[cin_k], outs=[cout_k])
                nc.gpsimd.collective_compute(
                    "AllGather", mybir.AluOpType.bypass,
                    replica_groups=[[0, 1], [2, 3], [4, 5], [6, 7]],
                    ins=[cin_v], outs=[cout_v])
                for jt in range(NT):
                    for g in range(2):
                        nc.sync.dma_start(
                            out=krot[jt][:, g * SQ:(g + 1) * SQ],
                            in_=cout_k[g * D + jt * 128:g * D + (jt + 1) * 128, :])
                for st in range(NT):
                    nc.scalar.dma_start(
                        out=vaug[st],
                        in_=cout_v[st * 128:(st + 1) * 128, :])

            # ---------------- phase 2: attention + o_proj ----------------
            with tc.tile_pool(name="pps", bufs=2, space="PSUM") as pps, \
                 tc.tile_pool(name="ppav", bufs=4, space="PSUM") as ppav, \
                 tc.tile_pool(name="ebp", bufs=6) as ebp, \
                 tc.tile_pool(name="pp", bufs=6) as ppool, \
                 tc.tile_pool(name="rp", bufs=4) as rp:
                import concourse.bass as _b
                for hpp in range(0, H // 2, 2):
                    av = {}
                    for hp in (hpp, hpp + 1):
                        for tag_h in (2 * hp, 2 * hp + 1):
                            av[tag_h] = ppav.tile([HD + 1, SQ], f32, tag="av", name=f"av{tag_h}")
                    for ktp in range(4):
                        for hp in (hpp, hpp + 1):
                            for h, base in ((2 * hp, 0), (2 * hp + 1, 64)):
                                pss = pps.tile([128, 2, 512], f32, tag="ss")
                                ebt = ebp.tile([128, 2, 512], bf16, tag="eb")
                                pm = ppool.tile([128, 2, 512], bf16, tag="pm")
                                for sub in range(2):
                                    kt = 2 * ktp + sub
                                    ks = slice(kt * 128, (kt + 1) * 128)
                                    nc.tensor.matmul(
                                        pss[:, sub, :],
                                        lhsT=krot[hp][base:base + 64, ks],
                                        rhs=qrot[hp][base:base + 64, :],
                                        start=True, stop=True)
                                    eng = nc.sync if (ktp + sub) % 2 == 0 else nc.scalar
                                    eng.dma_start(out=ebt[:, sub, :], in_=eb_h[h, ks, :])
                                nc.scalar.activation(out=pm, in_=pss, func=AF.Exp)
                                nc.vector.tensor_mul(out=pm, in0=pm, in1=ebt)
                                for sub in range(2):
                                    kt = 2 * ktp + sub
                                    va = _b.AP(
                                        tensor=vaug[kt].tensor,
                                        offset=vaug[kt].offset + h * (HD + 1),
                                        ap=[vaug[kt].ap[0], [1, HD + 1]],
                                    )
                                    nc.tensor.matmul(
                                        av[h], lhsT=va, rhs=pm[:, sub, :],
                                        start=(ktp == 0 and sub == 0),
                                        stop=(ktp == 3 and sub == 1))
                    for hp in (hpp, hpp + 1):
                        rbc = rp.tile([128, SQ], f32, tag="rbc")
                        for h, base in ((2 * hp, 0), (2 * hp + 1, 64)):
                            den1 = rp.tile([1, SQ], f32, tag="rden")
                            nc.scalar.copy(out=den1, in_=av[h][HD:HD + 1, :])
                            rden1 = rp.tile([1, SQ], f32, tag="rden")
                            nc.vector.reciprocal_approx_fast(out=rden1, in_=den1)
                            nc.gpsimd.dma_start(out=dscr[h:h + 1, :], in_=rden1)
                            bsrc = _b.AP(tensor=dscr.tensor, offset=h * SQ,
                                         ap=[[0, 64], [1, SQ]])
                            nc.gpsimd.dma_start(out=rbc[base:base + 64, :], in_=bsrc)
                            nc.vector.tensor_mul(out=outT[hp][base:base + 64, :],
                                                 in0=av[h][0:HD, :],
                                                 in1=rbc[base:base + 64, :])

            # ---------------- phase 3: o_proj + metric ----------------
            with tc.tile_pool(name="ppo", bufs=2, space="PSUM") as ppo, \
                 tc.tile_pool(name="ppm", bufs=2, space="PSUM") as ppm, \
                 tc.tile_pool(name="st3", bufs=2) as st3:
                for qs in range(4):
                    pof = ppo.tile([128, 2, 512], f32, tag="of")
                    for dt in range(NT):
                        for nh in range(2):
                            nc.tensor.matmul(
                                pof[:, nh, :],
                                lhsT=outT[dt][:, qs * 128:(qs + 1) * 128],
                                rhs=wo[dt][:, nh * 512:(nh + 1) * 512],
                                start=(dt == 0), stop=(dt == NT - 1))
                    stage = st3.tile([128, D], f32, tag="stage")
                    nc.scalar.copy(out=stage, in_=pof.rearrange("p a b -> p (a b)"))
                    nc.sync.dma_start(out=outp_h[qs * 128:(qs + 1) * 128, :], in_=stage)

                mstage = st3.tile([HD, S], f32, tag="mstage", name="mstage")
                for ch in range(2):
                    cs = slice(ch * 512, (ch + 1) * 512)
                    psm = ppm.tile([HD, 512], f32, tag="m")
                    for jt in range(NT):
                        nc.tensor.matmul(psm, lhsT=mpat, rhs=krot[jt][:, cs],
                                         start=(jt == 0), stop=(jt == NT - 1))
                    nc.scalar.copy(out=mstage[:, cs], in_=psm)
                nc.sync.dma_start(out=metT_h, in_=mstage)

    nc.compile()
    return nc


def _host_prep(inputs):
    x = np.asarray(inputs["x"], dtype=np.float32)
    attn_bias = np.asarray(inputs["attn_bias"], dtype=np.float32)
    pos_ids = np.asarray(inputs["pos_ids"])
    pad_mask = np.asarray(inputs["pad_mask"])
    Wq = np.asarray(inputs["Wq"], dtype=np.float32)
    Wk = np.asarray(inputs["Wk"], dtype=np.float32)
    Wv = np.asarray(inputs["Wv"], dtype=np.float32)
    Wo = np.asarray(inputs["Wo"], dtype=np.float32)

    wq_t = np.ascontiguousarray(Wq.T).astype(BF)
    wk_t = np.ascontiguousarray(Wk.T).astype(BF)
    wv_t = np.ascontiguousarray(Wv.T).astype(BF)
    wo_t = np.ascontiguousarray(Wo.T).astype(BF)

    # rotary tables per batch: cos/sin[s, hd] tiled twice over partitions
    inv = 1.0 / (10000.0 ** (np.arange(0, HD, 2, dtype=np.float32) / HD))  # [32]
    hd_idx = np.arange(64) % 32
    sign = np.where(np.arange(64) < 32, -1.0, 1.0).astype(np.float32)

    mpat = np.zeros((128, HD), dtype=np.float32)
    for p in range(128):
        mpat[p, p % 64] = 1.0 / H
    mpat = mpat.astype(BF)

    in_maps = []
    for c in range(NCORES):
        b, qc = divmod(c, 2)
        qs, qe = qc * SQ, (qc + 1) * SQ
        pos = pos_ids[b].astype(np.float32)  # [S]
        ang = pos[None, :] * inv[hd_idx][:, None]  # [64, S]
        cosf, sinf = np.cos(ang), np.sin(ang) * sign[:, None]  # [64, S]
        ckf = np.tile(cosf, (2, 1)).astype(np.float32)  # [128, S]
        skf = np.tile(sinf, (2, 1)).astype(np.float32)
        ck = np.ascontiguousarray(ckf[:, qs:qe])
        sk = np.ascontiguousarray(skf[:, qs:qe])
        cq = ck / 8.0
        sq = sk / 8.0

        eb = np.exp(attn_bias[b, :, qs:qe, :].transpose(0, 2, 1))  # [H, S_k, SQ]
        eb[:, pad_mask[b], :] = 0.0
        eb = np.ascontiguousarray(eb).astype(BF)

        xqb = np.ascontiguousarray(x[b].T[:, qs:qe]).astype(BF)  # [D, SQ]
        in_maps.append({
            "xq": xqb,
            "wq": wq_t, "wk": wk_t, "wv": wv_t, "wo": wo_t,
            "cq": cq, "sq": sq, "ck": ck, "sk": sk,
            "eb": eb, "mpat": mpat,
        })
    return in_maps


def kernel(**inputs):
    global _last_results
    from concourse.bass_utils import run_bass_kernel_spmd

    if "prog" not in _prog_cache:
        _prog_cache["prog"] = _build_program()
    nc = _prog_cache["prog"]

    in_maps = _host_prep(inputs)
    trace = os.environ.get("BASS_KERNEL_TRACE", "0") == "1"
    res = run_bass_kernel_spmd(nc, in_maps, core_ids=list(range(NCORES)),
                               trace=trace)
    _last_results = res

    bo = np.asarray(inputs["bo"], dtype=np.float32)
    out = np.empty((B, S, D), dtype=np.float32)
    metric = np.empty((B, S, HD), dtype=np.float32)
    for c in range(NCORES):
        b, qc = divmod(c, 2)
        out[b, qc * SQ:(qc + 1) * SQ, :] = res.results[c]["outp"]
        if qc == 0:
            metric[b] = res.results[c]["metT"].T
    out += bo[None, None, :]
    return out, metric


# revision 16
# speedup vs baseline: 1.0080x; 1.0080x over previous
"""Multi-head self-attention (RoPE + bias + pad-mask + head-mean metric) on 8 TRN2 cores.

Sharding: (batch, query-half) -> one core each: core c handles batch c//2,
queries [c%2*512, (c%2+1)*512). No collectives; host does pure slicing/concat.

Device layout (per core):
  - projections computed transposed: qT/kT [j, s] via lhsT=W^T-tile, rhs=x^T
  - rotary via "swap projection": second matmul chain against sign-permuted
    weight columns, then 2 muls (cos/sin tables) + add on DVE
  - scores computed transposed [k, q] (c=64 matmuls), softmax without
    max-subtraction (scores are bounded), pad mask + attn bias applied
    multiplicatively: p = exp(scores) * eb where eb = exp(bias) (0 if padded),
    streamed as bf16 via DMA-transpose
  - denominators via ones-column augmented V (row 64 of the AV psum), divided
    in via reciprocal + DRAM-round-trip partition broadcast
  - o_proj directly from the transposed attention output (lhsT), metric
    (head-mean of k_rot) via a constant averaging matrix on the PE
"""

import os
import numpy as np
import ml_dtypes

B, S, D, H, HD = 4, 1024, 1024, 16, 64
SQ = S // 2          # queries per core
NCORES = 8
NT = D // 128        # contraction tiles
BF = ml_dtypes.bfloat16

_prog_cache = {}
_last_results = None  # BassKernelResults of the most recent run (for test.py)


def _build_program():
    import concourse.bass as bass
    import concourse.tile as tile
    from concourse import bacc, mybir

    f32 = mybir.dt.float32
    bf16 = mybir.dt.bfloat16
    AF = mybir.ActivationFunctionType

    nc = bacc.Bacc("TRN2", target_bir_lowering=False, debug=False)

    def din(name, shape, dt=bf16):
        return nc.dram_tensor(name, shape, dt, kind="ExternalInput").ap()

    xT_h = din("xT", [D, S])
    xq_h = din("xq", [D, SQ])
    wq_h = din("wq", [D, D])
    wk_h = din("wk", [D, D])
    wv_h = din("wv", [D, D]); wo_h = din("wo", [D, D])
    cq_h = din("cq", [128, SQ], f32); sq_h = din("sq", [128, SQ], f32)
    ck_h = din("ck", [128, S], f32); sk_h = din("sk", [128, S], f32)
    eb_h = din("eb", [H, S, SQ])
    mpat_h = din("mpat", [128, HD])

    outp_h = nc.dram_tensor("outp", [SQ, D], f32, kind="ExternalOutput").ap()
    metT_h = nc.dram_tensor("metT", [HD, S], f32, kind="ExternalOutput").ap()

    dscr = nc.dram_tensor("dscr", [H, SQ], f32).ap()  # rden bounce buffer

    with tile.TileContext(nc) as tc:
        with tc.tile_pool(name="keep", bufs=1) as keep:
            # persistent across phases
            wo = [keep.tile([128, D], bf16, tag=f"wo{i}", name=f"wo{i}") for i in range(NT)]
            qrot = [keep.tile([128, SQ], bf16, tag=f"qr{i}", name=f"qr{i}") for i in range(NT)]
            krot = [keep.tile([128, S], bf16, tag=f"kr{i}", name=f"kr{i}") for i in range(NT)]
            vaug = [keep.tile([128, H * (HD + 1)], bf16, tag=f"va{i}", name=f"va{i}") for i in range(NT)]
            outT = [keep.tile([128, SQ], bf16, tag=f"ot{i}", name=f"ot{i}") for i in range(NT)]
            mpat = keep.tile([128, HD], bf16, tag="mpat")
            nc.sync.dma_start(out=mpat, in_=mpat_h)
            for i in range(NT):
                nc.sync.dma_start(out=wo[i], in_=wo_h[i * 128:(i + 1) * 128, :])

            # ---------------- phase 1: projections + rotary ----------------
            with tc.tile_pool(name="ph1", bufs=1) as ph1, \
                 tc.tile_pool(name="t1", bufs=4) as t1p, \
                 tc.tile_pool(name="ppq", bufs=2, space="PSUM") as ppq, \
                 tc.tile_pool(name="ppk", bufs=2, space="PSUM") as ppk, \
                 tc.tile_pool(name="ppv", bufs=1, space="PSUM") as ppv:
                xt = [ph1.tile([128, S], bf16, tag=f"xt{i}", name=f"xt{i}") for i in range(NT)]
                xq = [ph1.tile([128, SQ], bf16, tag=f"xq{i}", name=f"xq{i}") for i in range(NT)]
                wq = [ph1.tile([128, D], bf16, tag=f"wq{i}", name=f"wq{i}") for i in range(NT)]
                wk = [ph1.tile([128, D], bf16, tag=f"wk{i}", name=f"wk{i}") for i in range(NT)]
                wv = [ph1.tile([128, D], bf16, tag=f"wv{i}", name=f"wv{i}") for i in range(NT)]
                for i in range(NT):
                    sl = slice(i * 128, (i + 1) * 128)
                    nc.sync.dma_start(out=xq[i], in_=xq_h[sl, :])
                    nc.sync.dma_start(out=wq[i], in_=wq_h[sl, :])
                    nc.scalar.dma_start(out=xt[i], in_=xT_h[sl, :])
                    nc.scalar.dma_start(out=wk[i], in_=wk_h[sl, :])
                    nc.gpsimd.dma_start(out=wv[i], in_=wv_h[sl, :])
                import concourse.bass as _bass
                cq = ph1.tile([128, SQ], f32, tag="cq")
                sq = ph1.tile([128, SQ], f32, tag="sq")
                ck = ph1.tile([128, S], f32, tag="ck")
                sk = ph1.tile([128, S], f32, tag="sk")
                nc.sync.dma_start(out=cq, in_=cq_h)
                nc.sync.dma_start(out=sq, in_=sq_h)
                nc.sync.dma_start(out=ck, in_=ck_h)
                nc.sync.dma_start(out=sk, in_=sk_h)

                def v_tile(st):
                    ss = slice(st * 128, (st + 1) * 128)
                    ones_ap = _bass.AP(
                        tensor=vaug[st].tensor,
                        offset=vaug[st].offset + HD,
                        ap=[vaug[st].ap[0], [HD + 1, H], [1, 1]],
                    )
                    nc.vector.memset(ones_ap, 1.0)
                    psv = ppv.tile([128, 2, 512], f32, tag="psv", name=f"psv{st}")
                    for dt in range(NT):
                        for jh in range(2):
                            nc.tensor.matmul(psv[:, jh, :], lhsT=xt[dt][:, ss],
                                             rhs=wv[dt][:, jh * 512:(jh + 1) * 512],
                                             start=(dt == 0), stop=(dt == NT - 1))
                    dest = _bass.AP(
                        tensor=vaug[st].tensor,
                        offset=vaug[st].offset,
                        ap=[vaug[st].ap[0], [HD + 1, H], [1, HD]],
                    )
                    nc.scalar.copy(out=dest, in_=psv.rearrange("p c b -> p (c b)").rearrange("p (a b) -> p a b", a=H))

                def swap_dma(dst, src, width):
                    # dst[p] = src[p+-32 within each 64 block], via 4 shifted DMAs
                    for i, (a, bb) in enumerate(((0, 32), (32, 0), (64, 96), (96, 64))):
                        eng = nc.scalar if i % 2 == 0 else nc.gpsimd
                        eng.dma_start(out=dst[a:a + 32, :width],
                                      in_=src[bb:bb + 32, :width])

                for jt in range(NT):
                    js = slice(jt * 128, (jt + 1) * 128)
                    # q projection (m=j-tile, n=q), rotary via evac + swap-DMA
                    psq = ppq.tile([128, SQ], f32, tag="psq")
                    for dt in range(NT):
                        nc.tensor.matmul(psq, lhsT=wq[dt][:, js], rhs=xq[dt],
                                         start=(dt == 0), stop=(dt == NT - 1))
                    qsb = t1p.tile([128, S], f32, tag="tmp")
                    qsw = t1p.tile([128, S], f32, tag="tmp")
                    nc.scalar.copy(out=qsb[:, :SQ], in_=psq)
                    swap_dma(qsw, qsb, SQ)
                    t1 = t1p.tile([128, S], f32, tag="tmp")
                    t2 = t1p.tile([128, S], f32, tag="tmp")
                    nc.vector.tensor_mul(out=t1[:, :SQ], in0=qsb[:, :SQ], in1=cq)
                    nc.vector.tensor_mul(out=t2[:, :SQ], in0=qsw[:, :SQ], in1=sq)
                    nc.vector.tensor_add(out=qrot[jt], in0=t1[:, :SQ], in1=t2[:, :SQ])

                    # k projection (n=full S in two 512 chunks)
                    psk = ppk.tile([128, S], f32, tag="psk")
                    for dt in range(NT):
                        for nh in range(2):
                            ns = slice(nh * 512, (nh + 1) * 512)
                            nc.tensor.matmul(psk[:, ns], lhsT=wk[dt][:, js],
                                             rhs=xt[dt][:, ns],
                                             start=(dt == 0), stop=(dt == NT - 1))
                    ksb = t1p.tile([128, S], f32, tag="tmp")
                    ksw = t1p.tile([128, S], f32, tag="tmp")
                    nc.scalar.copy(out=ksb, in_=psk)
                    swap_dma(ksw, ksb, S)
                    t3 = t1p.tile([128, S], f32, tag="tmp")
                    t4 = t1p.tile([128, S], f32, tag="tmp")
                    nc.vector.tensor_mul(out=t3, in0=ksb, in1=ck)
                    nc.vector.tensor_mul(out=t4, in0=ksw, in1=sk)
                    nc.vector.tensor_add(out=krot[jt], in0=t3, in1=t4)

                    v_tile(jt)

            # ---------------- phase 2: attention + o_proj ----------------
            with tc.tile_pool(name="pps", bufs=2, space="PSUM") as pps, \
                 tc.tile_pool(name="ppav", bufs=4, space="PSUM") as ppav, \
                 tc.tile_pool(name="ebp", bufs=6) as ebp, \
                 tc.tile_pool(name="pp", bufs=6) as ppool, \
                 tc.tile_pool(name="rp", bufs=4) as rp:
                import concourse.bass as _b
                for hpp in range(0, H // 2, 2):
                    av = {}
                    for hp in (hpp, hpp + 1):
                        for tag_h in (2 * hp, 2 * hp + 1):
                            av[tag_h] = ppav.tile([HD + 1, SQ], f32, tag="av", name=f"av{tag_h}")
                    for ktp in range(4):
                        for hp in (hpp, hpp + 1):
                            for h, base in ((2 * hp, 0), (2 * hp + 1, 64)):
                                pss = pps.tile([128, 2, 512], f32, tag="ss")
                                ebt = ebp.tile([128, 2, 512], bf16, tag="eb")
                                pm = ppool.tile([128, 2, 512], bf16, tag="pm")
                                for sub in range(2):
                                    kt = 2 * ktp + sub
                                    ks = slice(kt * 128, (kt + 1) * 128)
                                    nc.tensor.matmul(
                                        pss[:, sub, :],
                                        lhsT=krot[hp][base:base + 64, ks],
                                        rhs=qrot[hp][base:base + 64, :],
                                        start=True, stop=True)
                                    eng = nc.sync if (ktp + sub) % 2 == 0 else nc.scalar
                                    eng.dma_start(out=ebt[:, sub, :], in_=eb_h[h, ks, :])
                                nc.scalar.activation(out=pm, in_=pss, func=AF.Exp)
                                nc.vector.tensor_mul(out=pm, in0=pm, in1=ebt)
                                for sub in range(2):
                                    kt = 2 * ktp + sub
                                    va = _b.AP(
                                        tensor=vaug[kt].tensor,
                                        offset=vaug[kt].offset + h * (HD + 1),
                                        ap=[vaug[kt].ap[0], [1, HD + 1]],
                                    )
                                    nc.tensor.matmul(
                                        av[h], lhsT=va, rhs=pm[:, sub, :],
                                        start=(ktp == 0 and sub == 0),
                                        stop=(ktp == 3 and sub == 1))
                    for hp in (hpp, hpp + 1):
                        rbc = rp.tile([128, SQ], f32, tag="rbc")
                        for h, base in ((2 * hp, 0), (2 * hp + 1, 64)):
                            den1 = rp.tile([1, SQ], f32, tag="rden")
                            nc.scalar.copy(out=den1, in_=av[h][HD:HD + 1, :])
                            rden1 = rp.tile([1, SQ], f32, tag="rden")
                            nc.vector.reciprocal_approx_fast(out=rden1, in_=den1)
                            nc.gpsimd.dma_start(out=dscr[h:h + 1, :], in_=rden1)
                            bsrc = _b.AP(tensor=dscr.tensor, offset=h * SQ,
                                         ap=[[0, 64], [1, SQ]])
                            nc.gpsimd.dma_start(out=rbc[base:base + 64, :], in_=bsrc)
                            nc.vector.tensor_mul(out=outT[hp][base:base + 64, :],
                                                 in0=av[h][0:HD, :],
                                                 in1=rbc[base:base + 64, :])

            # ---------------- phase 3: o_proj + metric ----------------
            with tc.tile_pool(name="ppo", bufs=2, space="PSUM") as ppo, \
                 tc.tile_pool(name="ppm", bufs=2, space="PSUM") as ppm, \
                 tc.tile_pool(name="st3", bufs=2) as st3:
                for qs in range(4):
                    pof = ppo.tile([128, 2, 512], f32, tag="of")
                    for dt in range(NT):
                        for nh in range(2):
                            nc.tensor.matmul(
                                pof[:, nh, :],
                                lhsT=outT[dt][:, qs * 128:(qs + 1) * 128],
                                rhs=wo[dt][:, nh * 512:(nh + 1) * 512],
                                start=(dt == 0), stop=(dt == NT - 1))
                    stage = st3.tile([128, D], f32, tag="stage")
                    nc.scalar.copy(out=stage, in_=pof.rearrange("p a b -> p (a b)"))
                    nc.sync.dma_start(out=outp_h[qs * 128:(qs + 1) * 128, :], in_=stage)

                mstage = st3.tile([HD, S], f32, tag="mstage", name="mstage")
                for ch in range(2):
                    cs = slice(ch * 512, (ch + 1) * 512)
                    psm = ppm.tile([HD, 512], f32, tag="m")
                    for jt in range(NT):
                        nc.tensor.matmul(psm, lhsT=mpat, rhs=krot[jt][:, cs],
                                         start=(jt == 0), stop=(jt == NT - 1))
                    nc.scalar.copy(out=mstage[:, cs], in_=psm)
                nc.sync.dma_start(out=metT_h, in_=mstage)

    nc.compile()
    return nc


def _host_prep(inputs):
    x = np.asarray(inputs["x"], dtype=np.float32)
    attn_bias = np.asarray(inputs["attn_bias"], dtype=np.float32)
    pos_ids = np.asarray(inputs["pos_ids"])
    pad_mask = np.asarray(inputs["pad_mask"])
    Wq = np.asarray(inputs["Wq"], dtype=np.float32)
    Wk = np.asarray(inputs["Wk"], dtype=np.float32)
    Wv = np.asarray(inputs["Wv"], dtype=np.float32)
    Wo = np.asarray(inputs["Wo"], dtype=np.float32)

    wq_t = np.ascontiguousarray(Wq.T).astype(BF)
    wk_t = np.ascontiguousarray(Wk.T).astype(BF)
    wv_t = np.ascontiguousarray(Wv.T).astype(BF)
    wo_t = np.ascontiguousarray(Wo.T).astype(BF)

    # rotary tables per batch: cos/sin[s, hd] tiled twice over partitions
    inv = 1.0 / (10000.0 ** (np.arange(0, HD, 2, dtype=np.float32) / HD))  # [32]
    hd_idx = np.arange(64) % 32
    sign = np.where(np.arange(64) < 32, -1.0, 1.0).astype(np.float32)

    mpat = np.zeros((128, HD), dtype=np.float32)
    for p in range(128):
        mpat[p, p % 64] = 1.0 / H
    mpat = mpat.astype(BF)

    in_maps = []
    for c in range(NCORES):
        b, qc = divmod(c, 2)
        qs, qe = qc * SQ, (qc + 1) * SQ
        pos = pos_ids[b].astype(np.float32)  # [S]
        ang = pos[None, :] * inv[hd_idx][:, None]  # [64, S]
        cosf, sinf = np.cos(ang), np.sin(ang) * sign[:, None]  # [64, S]
        ck = np.tile(cosf, (2, 1)).astype(np.float32)  # [128, S]
        sk = np.tile(sinf, (2, 1)).astype(np.float32)
        cq = (ck[:, qs:qe] / 8.0).copy()
        sq = (sk[:, qs:qe] / 8.0).copy()

        eb = np.exp(attn_bias[b, :, qs:qe, :].transpose(0, 2, 1))  # [H, S_k, SQ]
        eb[:, pad_mask[b], :] = 0.0
        eb = np.ascontiguousarray(eb).astype(BF)

        xTb = np.ascontiguousarray(x[b].T).astype(BF)  # [D, S]
        in_maps.append({
            "xT": xTb, "xq": np.ascontiguousarray(xTb[:, qs:qe]),
            "wq": wq_t, "wk": wk_t, "wv": wv_t, "wo": wo_t,
            "cq": cq, "sq": sq, "ck": ck, "sk": sk,
            "eb": eb, "mpat": mpat,
        })
    return in_maps


def kernel(**inputs):
    global _last_results
    from concourse.bass_utils import run_bass_kernel_spmd

    if "prog" not in _prog_cache:
        _prog_cache["prog"] = _build_program()
    nc = _prog_cache["prog"]

    in_maps = _host_prep(inputs)
    trace = os.environ.get("BASS_KERNEL_TRACE", "0") == "1"
    res = run_bass_kernel_spmd(nc, in_maps, core_ids=list(range(NCORES)),
                               trace=trace)
    _last_results = res

    bo = np.asarray(inputs["bo"], dtype=np.float32)
    out = np.empty((B, S, D), dtype=np.float32)
    metric = np.empty((B, S, HD), dtype=np.float32)
    for c in range(NCORES):
        b, qc = divmod(c, 2)
        out[b, qc * SQ:(qc + 1) * SQ, :] = res.results[c]["outp"]
        if qc == 0:
            metric[b] = res.results[c]["metT"].T
    out += bo[None, None, :]
    return out, metric


# revision 29
# speedup vs baseline: 1.3029x; 1.2926x over previous
"""Multi-head self-attention (RoPE + bias + pad-mask + head-mean metric) on 8 TRN2 cores.

Sharding: (batch, query-half) -> one core each: core c handles batch c//2,
queries [c%2*512, (c%2+1)*512). No collectives; host does pure slicing/concat.

Device layout (per core):
  - projections computed transposed: qT/kT [j, s] via lhsT=W^T-tile, rhs=x^T
  - rotary via "swap projection": second matmul chain against sign-permuted
    weight columns, then 2 muls (cos/sin tables) + add on DVE
  - scores computed transposed [k, q] (c=64 matmuls), softmax without
    max-subtraction (scores are bounded), pad mask + attn bias applied
    multiplicatively: p = exp(scores) * eb where eb = exp(bias) (0 if padded),
    streamed as bf16 via DMA-transpose
  - denominators via ones-column augmented V (row 64 of the AV psum), divided
    in via reciprocal + DRAM-round-trip partition broadcast
  - o_proj directly from the transposed attention output (lhsT), metric
    (head-mean of k_rot) via a constant averaging matrix on the PE
"""

import os
import numpy as np
import ml_dtypes

B, S, D, H, HD = 4, 1024, 1024, 16, 64
SQ = S // 2          # queries per core
NCORES = 8
NT = D // 128        # contraction tiles
BF = ml_dtypes.bfloat16

_prog_cache = {}
_last_results = None  # BassKernelResults of the most recent run (for test.py)


def _build_program():
    import concourse.bass as bass
    import concourse.tile as tile
    from concourse import bacc, mybir

    f32 = mybir.dt.float32
    bf16 = mybir.dt.bfloat16
    AF = mybir.ActivationFunctionType

    nc = bacc.Bacc("TRN2", target_bir_lowering=False, debug=False)

    def din(name, shape, dt=bf16):
        return nc.dram_tensor(name, shape, dt, kind="ExternalInput").ap()

    xT_h = din("xT", [D, S])
    xq_h = din("xq", [D, SQ])
    wq_h = din("wq", [D, D])
    wk_h = din("wk", [D, D])
    wv_h = din("wv", [D, D]); wo_h = din("wo", [D, D])
    cq_h = din("cq", [128, SQ]); sq_h = din("sq", [128, SQ])
    ck_h = din("ck", [128, S]); sk_h = din("sk", [128, S])
    eb_h = din("eb", [H, S, SQ])
    mpat_h = din("mpat", [128, HD])

    outp_h = nc.dram_tensor("outp", [SQ, D], f32, kind="ExternalOutput").ap()
    metT_h = nc.dram_tensor("metT", [HD, S], f32, kind="ExternalOutput").ap()

    dscr = nc.dram_tensor("dscr", [H, SQ], f32).ap()  # rden bounce buffer

    with tile.TileContext(nc) as tc:
        with tc.tile_pool(name="keep", bufs=1) as keep:
            # persistent across phases
            wo = [keep.tile([128, D], bf16, tag=f"wo{i}", name=f"wo{i}") for i in range(NT)]
            qrot = [keep.tile([128, SQ], bf16, tag=f"qr{i}", name=f"qr{i}") for i in range(NT)]
            krot = [keep.tile([128, S], bf16, tag=f"kr{i}", name=f"kr{i}") for i in range(NT)]
            vaug = [keep.tile([128, H * (HD + 1)], bf16, tag=f"va{i}", name=f"va{i}") for i in range(NT)]
            outT = [keep.tile([128, SQ], bf16, tag=f"ot{i}", name=f"ot{i}") for i in range(NT)]
            mpat = keep.tile([128, HD], bf16, tag="mpat")
            nc.sync.dma_start(out=mpat, in_=mpat_h)
            for i in range(NT):
                nc.gpsimd.dma_start(out=wo[i], in_=wo_h[i * 128:(i + 1) * 128, :])

            # ---------------- phase 1: projections + rotary ----------------
            with tc.tile_pool(name="ph1", bufs=1) as ph1, \
                 tc.tile_pool(name="t1", bufs=8) as t1p, \
                 tc.tile_pool(name="ppq", bufs=2, space="PSUM") as ppq, \
                 tc.tile_pool(name="ppk", bufs=2, space="PSUM") as ppk, \
                 tc.tile_pool(name="ppv", bufs=1, space="PSUM") as ppv:
                xt = [ph1.tile([128, S], bf16, tag=f"xt{i}", name=f"xt{i}") for i in range(NT)]
                xq = [ph1.tile([128, SQ], bf16, tag=f"xq{i}", name=f"xq{i}") for i in range(NT)]
                wq = [ph1.tile([128, D], bf16, tag=f"wq{i}", name=f"wq{i}") for i in range(NT)]
                wk = [ph1.tile([128, D], bf16, tag=f"wk{i}", name=f"wk{i}") for i in range(NT)]
                wv = [ph1.tile([128, D], bf16, tag=f"wv{i}", name=f"wv{i}") for i in range(NT)]
                for i in range(NT):
                    sl = slice(i * 128, (i + 1) * 128)
                    nc.sync.dma_start(out=xq[i], in_=xq_h[sl, :])
                    nc.scalar.dma_start(out=wq[i], in_=wq_h[sl, :])
                for i in range(NT):
                    sl = slice(i * 128, (i + 1) * 128)
                    nc.sync.dma_start(out=xt[i], in_=xT_h[sl, :])
                    nc.scalar.dma_start(out=wk[i], in_=wk_h[sl, :])
                    nc.gpsimd.dma_start(out=wv[i], in_=wv_h[sl, :])
                import concourse.bass as _bass
                cq = ph1.tile([128, SQ], bf16, tag="cq")
                sq = ph1.tile([128, SQ], bf16, tag="sq")
                ck = ph1.tile([128, S], bf16, tag="ck")
                sk = ph1.tile([128, S], bf16, tag="sk")
                nc.sync.dma_start(out=cq, in_=cq_h)
                nc.sync.dma_start(out=sq, in_=sq_h)
                nc.sync.dma_start(out=ck, in_=ck_h)
                nc.sync.dma_start(out=sk, in_=sk_h)

                def v_tile(st):
                    ss = slice(st * 128, (st + 1) * 128)
                    ones_ap = _bass.AP(
                        tensor=vaug[st].tensor,
                        offset=vaug[st].offset + HD,
                        ap=[vaug[st].ap[0], [HD + 1, H], [1, 1]],
                    )
                    nc.vector.memset(ones_ap, 1.0)
                    psv = ppv.tile([128, 2, 512], f32, tag="psv", name=f"psv{st}")
                    for dt in range(NT):
                        for jh in range(2):
                            nc.tensor.matmul(psv[:, jh, :], lhsT=xt[dt][:, ss],
                                             rhs=wv[dt][:, jh * 512:(jh + 1) * 512],
                                             start=(dt == 0), stop=(dt == NT - 1))
                    dest = _bass.AP(
                        tensor=vaug[st].tensor,
                        offset=vaug[st].offset,
                        ap=[vaug[st].ap[0], [HD + 1, H], [1, HD]],
                    )
                    nc.scalar.copy(out=dest, in_=psv.rearrange("p c b -> p (c b)").rearrange("p (a b) -> p a b", a=H))

                def swap_dma(dst, src, width):
                    # dst[p] = src[p+-32 within each 64 block], via 4 shifted DMAs
                    for i, (a, bb) in enumerate(((0, 32), (32, 0), (64, 96), (96, 64))):
                        eng = nc.scalar if i % 2 == 0 else nc.gpsimd
                        eng.dma_start(out=dst[a:a + 32, :width],
                                      in_=src[bb:bb + 32, :width])

                for jt in range(NT):
                    js = slice(jt * 128, (jt + 1) * 128)
                    # q projection (m=j-tile, n=q), rotary via evac + swap-DMA
                    psq = ppq.tile([128, SQ], f32, tag="psq")
                    for dt in range(NT):
                        nc.tensor.matmul(psq, lhsT=wq[dt][:, js], rhs=xq[dt],
                                         start=(dt == 0), stop=(dt == NT - 1))
                    qsb = t1p.tile([128, S], bf16, tag="tmp")
                    qsw = t1p.tile([128, S], bf16, tag="tmp")
                    nc.scalar.copy(out=qsb[:, :SQ], in_=psq)
                    swap_dma(qsw, qsb, SQ)
                    t1 = t1p.tile([128, S], bf16, tag="tmp")
                    t2 = t1p.tile([128, S], bf16, tag="tmp")
                    nc.vector.tensor_mul(out=t1[:, :SQ], in0=qsb[:, :SQ], in1=cq)
                    nc.vector.tensor_mul(out=t2[:, :SQ], in0=qsw[:, :SQ], in1=sq)
                    nc.vector.tensor_add(out=qrot[jt], in0=t1[:, :SQ], in1=t2[:, :SQ])

                    # k projection (n=full S in two 512 chunks)
                    psk = ppk.tile([128, S], f32, tag="psk")
                    for dt in range(NT):
                        for nh in range(2):
                            ns = slice(nh * 512, (nh + 1) * 512)
                            nc.tensor.matmul(psk[:, ns], lhsT=wk[dt][:, js],
                                             rhs=xt[dt][:, ns],
                                             start=(dt == 0), stop=(dt == NT - 1))
                    ksb = t1p.tile([128, S], bf16, tag="tmp")
                    ksw = t1p.tile([128, S], bf16, tag="tmp")
                    nc.scalar.copy(out=ksb, in_=psk)
                    swap_dma(ksw, ksb, S)
                    t3 = t1p.tile([128, S], bf16, tag="tmp")
                    t4 = t1p.tile([128, S], bf16, tag="tmp")
                    nc.vector.tensor_mul(out=t3, in0=ksb, in1=ck)
                    nc.vector.tensor_mul(out=t4, in0=ksw, in1=sk)
                    nc.vector.tensor_add(out=krot[jt], in0=t3, in1=t4)

                    v_tile(jt)

            # ---------------- phase 2: attention + o_proj ----------------
            with tc.tile_pool(name="pps", bufs=2, space="PSUM") as pps, \
                 tc.tile_pool(name="ppav", bufs=4, space="PSUM") as ppav, \
                 tc.tile_pool(name="ebp", bufs=6) as ebp, \
                 tc.tile_pool(name="pp", bufs=6) as ppool, \
                 tc.tile_pool(name="rp", bufs=4) as rp:
                import concourse.bass as _b
                for hpp in range(0, H // 2, 2):
                    av = {}
                    for hp in (hpp, hpp + 1):
                        for tag_h in (2 * hp, 2 * hp + 1):
                            av[tag_h] = ppav.tile([HD + 1, SQ], f32, tag="av", name=f"av{tag_h}")
                    for ktp in range(4):
                        for hp in (hpp, hpp + 1):
                            for h, base in ((2 * hp, 0), (2 * hp + 1, 64)):
                                pss = pps.tile([128, 2, 512], f32, tag="ss")
                                ebt = ebp.tile([128, 2, 512], bf16, tag="eb")
                                pm = ppool.tile([128, 2, 512], bf16, tag="pm")
                                for sub in range(2):
                                    kt = 2 * ktp + sub
                                    ks = slice(kt * 128, (kt + 1) * 128)
                                    nc.tensor.matmul(
                                        pss[:, sub, :],
                                        lhsT=krot[hp][base:base + 64, ks],
                                        rhs=qrot[hp][base:base + 64, :],
                                        start=True, stop=True)
                                    eng = nc.sync if (ktp + sub) % 2 == 0 else nc.scalar
                                    eng.dma_start(out=ebt[:, sub, :], in_=eb_h[h, ks, :])
                                nc.scalar.activation(out=pm, in_=pss, func=AF.Exp)
                                nc.vector.tensor_mul(out=pm, in0=pm, in1=ebt)
                                for sub in range(2):
                                    kt = 2 * ktp + sub
                                    va = _b.AP(
                                        tensor=vaug[kt].tensor,
                                        offset=vaug[kt].offset + h * (HD + 1),
                                        ap=[vaug[kt].ap[0], [1, HD + 1]],
                                    )
                                    nc.tensor.matmul(
                                        av[h], lhsT=va, rhs=pm[:, sub, :],
                                        start=(ktp == 0 and sub == 0),
                                        stop=(ktp == 3 and sub == 1))
                    for hp in (hpp, hpp + 1):
                        rbc = rp.tile([128, SQ], f32, tag="rbc")
                        for h, base in ((2 * hp, 0), (2 * hp + 1, 64)):
                            den1 = rp.tile([1, SQ], f32, tag="rden")
                            nc.scalar.copy(out=den1, in_=av[h][HD:HD + 1, :])
                            rden1 = rp.tile([1, SQ], f32, tag="rden")
                            nc.vector.reciprocal_approx_fast(out=rden1, in_=den1)
                            nc.gpsimd.dma_start(out=dscr[h:h + 1, :], in_=rden1)
                            bsrc = _b.AP(tensor=dscr.tensor, offset=h * SQ,
                                         ap=[[0, 64], [1, SQ]])
                            nc.gpsimd.dma_start(out=rbc[base:base + 64, :], in_=bsrc)
                            nc.vector.tensor_mul(out=outT[hp][base:base + 64, :],
                                                 in0=av[h][0:HD, :],
                                                 in1=rbc[base:base + 64, :])

            # ---------------- phase 3: o_proj + metric ----------------
            with tc.tile_pool(name="ppo", bufs=2, space="PSUM") as ppo, \
                 tc.tile_pool(name="ppm", bufs=2, space="PSUM") as ppm, \
                 tc.tile_pool(name="st3", bufs=2) as st3:
                for qs in range(4):
                    pof = ppo.tile([128, 2, 512], f32, tag="of")
                    for dt in range(NT):
                        for nh in range(2):
                            nc.tensor.matmul(
                                pof[:, nh, :],
                                lhsT=outT[dt][:, qs * 128:(qs + 1) * 128],
                                rhs=wo[dt][:, nh * 512:(nh + 1) * 512],
                                start=(dt == 0), stop=(dt == NT - 1))
                    stage = st3.tile([128, D], f32, tag="stage")
                    nc.scalar.copy(out=stage, in_=pof.rearrange("p a b -> p (a b)"))
                    nc.sync.dma_start(out=outp_h[qs * 128:(qs + 1) * 128, :], in_=stage)

                mstage = st3.tile([HD, S], f32, tag="mstage", name="mstage")
                for ch in range(2):
                    cs = slice(ch * 512, (ch + 1) * 512)
                    psm = ppm.tile([HD, 512], f32, tag="m")
                    for jt in range(NT):
                        nc.tensor.matmul(psm, lhsT=mpat, rhs=krot[jt][:, cs],
                                         start=(jt == 0), stop=(jt == NT - 1))
                    nc.scalar.copy(out=mstage[:, cs], in_=psm)
                nc.sync.dma_start(out=metT_h, in_=mstage)

    nc.compile()
    return nc


def _host_prep(inputs):
    x = np.asarray(inputs["x"], dtype=np.float32)
    attn_bias = np.asarray(inputs["attn_bias"], dtype=np.float32)
    pos_ids = np.asarray(inputs["pos_ids"])
    pad_mask = np.asarray(inputs["pad_mask"])
    Wq = np.asarray(inputs["Wq"], dtype=np.float32)
    Wk = np.asarray(inputs["Wk"], dtype=np.float32)
    Wv = np.asarray(inputs["Wv"], dtype=np.float32)
    Wo = np.asarray(inputs["Wo"], dtype=np.float32)

    wq_t = np.ascontiguousarray(Wq.T).astype(BF)
    wk_t = np.ascontiguousarray(Wk.T).astype(BF)
    wv_t = np.ascontiguousarray(Wv.T).astype(BF)
    wo_t = np.ascontiguousarray(Wo.T).astype(BF)

    # rotary tables per batch: cos/sin[s, hd] tiled twice over partitions
    inv = 1.0 / (10000.0 ** (np.arange(0, HD, 2, dtype=np.float32) / HD))  # [32]
    hd_idx = np.arange(64) % 32
    sign = np.where(np.arange(64) < 32, -1.0, 1.0).astype(np.float32)

    mpat = np.zeros((128, HD), dtype=np.float32)
    for p in range(128):
        mpat[p, p % 64] = 1.0 / H
    mpat = mpat.astype(BF)

    in_maps = []
    for c in range(NCORES):
        b, qc = divmod(c, 2)
        qs, qe = qc * SQ, (qc + 1) * SQ
        pos = pos_ids[b].astype(np.float32)  # [S]
        ang = pos[None, :] * inv[hd_idx][:, None]  # [64, S]
        cosf, sinf = np.cos(ang), np.sin(ang) * sign[:, None]  # [64, S]
        ck = np.tile(cosf, (2, 1)).astype(np.float32)  # [128, S]
        sk = np.tile(sinf, (2, 1)).astype(np.float32)
        cq = (ck[:, qs:qe] / 8.0).copy()
        sq = (sk[:, qs:qe] / 8.0).copy()

        eb = np.exp(attn_bias[b, :, qs:qe, :].transpose(0, 2, 1))  # [H, S_k, SQ]
        eb[:, pad_mask[b], :] = 0.0
        eb = np.ascontiguousarray(eb).astype(BF)

        xTb = np.ascontiguousarray(x[b].T).astype(BF)  # [D, S]
        in_maps.append({
            "xT": xTb, "xq": np.ascontiguousarray(xTb[:, qs:qe]),
            "wq": wq_t, "wk": wk_t, "wv": wv_t, "wo": wo_t,
            "cq": cq.astype(BF), "sq": sq.astype(BF),
            "ck": ck.astype(BF), "sk": sk.astype(BF),
            "eb": eb, "mpat": mpat,
        })
    return in_maps


def kernel(**inputs):
    global _last_results
    from concourse.bass_utils import run_bass_kernel_spmd

    if "prog" not in _prog_cache:
        _prog_cache["prog"] = _build_program()
    nc = _prog_cache["prog"]

    in_maps = _host_prep(inputs)
    trace = os.environ.get("BASS_KERNEL_TRACE", "0") == "1"
    res = run_bass_kernel_spmd(nc, in_maps, core_ids=list(range(NCORES)),
                               trace=trace)
    _last_results = res

    bo = np.asarray(inputs["bo"], dtype=np.float32)
    out = np.empty((B, S, D), dtype=np.float32)
    metric = np.empty((B, S, HD), dtype=np.float32)
    for c in range(NCORES):
        b, qc = divmod(c, 2)
        out[b, qc * SQ:(qc + 1) * SQ, :] = res.results[c]["outp"]
        if qc == 0:
            metric[b] = res.results[c]["metT"].T
    out += bo[None, None, :]
    return out, metric
